# revision 3
# baseline (speedup 1.0000x reference)
"""Trainium2 Bass kernel for nn_Attention_9998683865539 (bf16 operands, baseline head order).

Multi-head attention (B=8, N=1024, C=768, H=12, HD=64, fp32), data-parallel
over the batch across 8 NeuronCores (one batch element per core, weights
replicated, no collectives).

v2 scheduling changes over the baseline:
  - DMA order: w_qk blocks 0,6 land first as single transfers, then x in
    per-ct chunks consumed immediately by interleaved qk0/qk6 matmuls, then
    blocks 1,7 (computed monolithically pre-head-0), then wv.
  - head(): U matmuls lag S by one k-tile so the PE never waits on the
    ACT exp; qk filler chunks sit between S(kt) and U(kt-1).
  - last head is split into two query halves; the output projection for
    the first half's token tiles is interleaved into the second half's
    pipeline, shrinking the serial tail.
"""
import sys

sys.path.insert(0, "/opt/trn_rl_repo")

import collections

import numpy as np

import concourse.bass as bass
import concourse.tile as tile
from concourse import bacc, mybir
from concourse import bass_utils

F32 = mybir.dt.float32
F32R = mybir.dt.float32r
BF16 = mybir.dt.bfloat16
EXP = mybir.ActivationFunctionType.Exp
MULT = mybir.AluOpType.mult

B = 8            # batch (one element per core)
C = 768          # channels
N = 1024         # tokens
H = 12           # heads
HD = 64          # head dim
SCALE = HD ** -0.5
NCT = C // 128   # 6 channel tiles
NTT = N // 128   # 8 token tiles
NQK = 12         # qk feature tiles (1536/128)
WV = H * (HD + 1)  # 780: per token-tile, 12 heads x (64 v + 1 ones)


def _build(reps=0, pt_bufs=6, wqs_bufs=8, lag=2):
    nc = bacc.Bacc("TRN2", target_bir_lowering=False, debug=False)

    xT_d = nc.dram_tensor("xT", [C, N], BF16, kind="ExternalInput").ap()
    wqb_d = nc.dram_tensor("wqb", [NQK, C, 128], BF16, kind="ExternalInput").ap()
    wv_d = nc.dram_tensor("wv", [C, C], BF16, kind="ExternalInput").ap()
    wp_d = nc.dram_tensor("wp", [C, C], BF16, kind="ExternalInput").ap()
    bp_d = nc.dram_tensor("bp", [128, C], F32, kind="ExternalInput").ap()
    out_d = nc.dram_tensor("out", [N, C], F32, kind="ExternalOutput").ap()

    with tile.TileContext(nc) as tc:
        with (
            tc.tile_pool(name="big", bufs=1) as big,
            tc.tile_pool(name="ptp", bufs=pt_bufs) as ptp,
            tc.tile_pool(name="wkp", bufs=1) as wkp,
            tc.tile_pool(name="psp", bufs=2, space=bass.MemorySpace.PSUM) as psp,
        ):
            qk_t = big.tile([128, NQK * N], BF16)     # 24KB/part
            vp_t = big.tile([128, NTT * WV], BF16)    # 12.2KB/part
            attnT = big.tile([128, NCT * N], BF16)    # 12KB/part
            xr = big.tile([128, NCT * N], BF16)       # 12KB/part
            wv_t = big.tile([128, NCT * C], BF16)     # 9KB/part
            wp_t = big.tile([128, NCT * C], BF16)     # 9KB/part
            ones12 = wkp.tile([128, H], BF16)
            bias_sb = wkp.tile([128, C], F32)
            warm = wkp.tile([128, 1], F32)

            LAG = lag

            def emit():
                # constants; dummy exp pulls the ACT table load (~2.7us)
                # into the DMA ramp instead of the first real exp
                nc.vector.memset(ones12[:], 1.0)
                nc.scalar.activation(warm[:], ones12[:, 0:1], EXP)
                ps_w = psp.tile([128, N], F32, tag="s", name="ps_warm")
                for _ in range(30):
                    nc.tensor.matmul(
                        ps_w[0:H, 0:H], ones12[:], ones12[:],
                        start=True, stop=True,
                    )

                def wq_load(ft, eng=None):
                    wqs = wkp.tile(
                        [128, NCT * 128], BF16, tag="wqs", bufs=wqs_bufs
                    )
                    (eng or nc.gpsimd).dma_start(
                        wqs[:].rearrange("p (ct f) -> p ct f", f=128),
                        wqb_d[ft].rearrange("(ct p) f -> p ct f", p=128),
                    )
                    return wqs

                # DMA order: w0, w6 (whole blocks), x per-ct chunks, w1, w7,
                # wv, bias; descriptor generation is spread across the SP /
                # DVE / ACT / Pool queues so the ramp is transfer-bound.
                w_0 = wq_load(0, nc.gpsimd)
                w_6 = wq_load(6, nc.gpsimd)
                xq = [nc.sync, nc.scalar, nc.sync, nc.scalar,
                      nc.sync, nc.scalar]
                for ct in range(NCT):
                    xq[ct].dma_start(
                        xr[:, N * ct : N * (ct + 1)],
                        xT_d[128 * ct : 128 * (ct + 1), :],
                    )
                w_1 = wq_load(1, nc.gpsimd)
                w_7 = wq_load(7, nc.gpsimd)
                nc.gpsimd.dma_start(
                    wv_t[:].rearrange("p (ct f) -> p ct f", f=C),
                    wv_d[:].rearrange("(ct p) f -> p ct f", p=128),
                )
                nc.sync.dma_start(bias_sb[:], bp_d[:])

                # qk blocks 0 and 6 interleaved per-ct: consume x chunks as
                # they arrive, accumulating in two PSUM s-slots.
                ps0 = psp.tile([128, N], F32, tag="s", name="ps_qk0")
                ps6 = psp.tile([128, N], F32, tag="s", name="ps_qk6")
                for ct in range(NCT):
                    for ps, wb in ((ps0, w_0), (ps6, w_6)):
                        lhs = wb[:, 128 * ct : 128 * (ct + 1)]
                        for qh in range(2):
                            nc.tensor.matmul(
                                ps[:, 512 * qh : 512 * (qh + 1)],
                                lhs,
                                xr[:, N * ct + 512 * qh : N * ct + 512 * (qh + 1)],
                                start=(ct == 0),
                                stop=(ct == NCT - 1),
                            )
                nc.vector.tensor_copy(qk_t[:, 0:N], ps0[:])
                nc.vector.tensor_copy(qk_t[:, 6 * N : 7 * N], ps6[:])


                def qk_compute(ft, wqs):
                    """qkT block ft, monolithic (pre-head phase)."""
                    ps = psp.tile([128, N], F32, tag="s")
                    for ct in range(NCT):
                        lhs = wqs[:, 128 * ct : 128 * (ct + 1)]
                        for qh in range(2):
                            nc.tensor.matmul(
                                ps[:, 512 * qh : 512 * (qh + 1)],
                                lhs,
                                xr[:, N * ct + 512 * qh : N * ct + 512 * (qh + 1)],
                                start=(ct == 0),
                                stop=(ct == NCT - 1),
                            )
                    nc.vector.tensor_copy(qk_t[:, N * ft : N * (ft + 1)], ps[:])

                filler = collections.deque()

                def queue_qk_chunks(ft, wqs):
                    """qkT block ft as 6 filler chunks (2 matmuls each),
                    accumulating in a u-tag PSUM slot."""
                    cell = {}

                    def chunk(ct):
                        if ct == 0:
                            cell["ps"] = psp.tile(
                                [128, N], F32, tag="u", name="qk_acc"
                            )
                        ps = cell["ps"]
                        lhs = wqs[:, 128 * ct : 128 * (ct + 1)]
                        for qh in range(2):
                            nc.tensor.matmul(
                                ps[:, 512 * qh : 512 * (qh + 1)],
                                lhs,
                                xr[:, N * ct + 512 * qh : N * ct + 512 * (qh + 1)],
                                start=(ct == 0),
                                stop=(ct == NCT - 1),
                            )
                        if ct == NCT - 1:
                            nc.vector.tensor_copy(
                                qk_t[:, N * ft : N * (ft + 1)], ps[:]
                            )

                    for ct in range(NCT):
                        filler.append((ft, lambda ct=ct: chunk(ct)))

                def v_block(m, tag="s"):
                    """v token-tile m -> vp [128, 780]: 12x(64 v cols + ones)."""
                    ps = psp.tile([128, N], F32, tag=tag, name="v_acc")
                    for ct in range(NCT):
                        lhs = xr[:, N * ct + 128 * m : N * ct + 128 * (m + 1)]
                        for nn, nw in ((0, 512), (512, 256)):
                            nc.tensor.matmul(
                                ps[:, nn : nn + nw],
                                lhs,
                                wv_t[:, C * ct + nn : C * ct + nn + nw],
                                start=(ct == 0),
                                stop=(ct == NCT - 1),
                            )
                    blk = vp_t[:, WV * m : WV * (m + 1)].rearrange(
                        "p (h c) -> p h c", c=HD + 1
                    )
                    nc.vector.tensor_copy(
                        blk[:, :, 0:HD],
                        ps[:, 0:C].rearrange("p (h c) -> p h c", c=HD),
                    )
                    nc.vector.tensor_copy(
                        blk[:, :, HD : HD + 1],
                        ones12[:].rearrange("p (h o) -> p h o", o=1),
                    )

                def normalize(po, qft, ps_u, n0, n1):
                    """Evacuate U+den for cols [n0:n1), normalize off the PE:
                    recip (DVE) -> partition_broadcast (gpsimd) -> mult."""
                    w = n1 - n0
                    uT = wkp.tile([128, N], F32, tag="uT", bufs=1)
                    nc.vector.tensor_copy(uT[0:65, 0:w], ps_u[0:65, n0:n1])
                    rec_f = wkp.tile([1, N], F32, tag="recf2", bufs=1)
                    nc.vector.reciprocal(rec_f[:, 0:w], uT[64:65, 0:w])
                    bc = wkp.tile([64, N], F32, tag="bc", bufs=1)
                    nc.gpsimd.partition_broadcast(bc[:, 0:w], rec_f[:, 0:w])
                    nc.vector.tensor_tensor(
                        attnT[po : po + 64, N * qft + n0 : N * qft + n1],
                        uT[0:64, 0:w],
                        bc[:, 0:w],
                        op=MULT,
                    )

                def head0_split():
                    """Head 0 in two waves of 4 k-tiles: scores+exp emitted
                    before that wave's v blocks, so ACT drains exps while the
                    PE computes v. Wave size matches pt_bufs."""
                    qft, po, kft = 0, 0, 6
                    wave = min(pt_bufs, 4)
                    ps_u = psp.tile([128, N], F32, tag="u")
                    for w0 in range(0, NTT, wave):
                        pts = []
                        for kt in range(w0, w0 + wave):
                            ps_s = psp.tile([128, N], F32, tag="s")
                            ksl = qk_t[
                                po : po + HD,
                                N * kft + 128 * kt : N * kft + 128 * (kt + 1),
                            ]
                            for qh in range(2):
                                nc.tensor.matmul(
                                    ps_s[:, 512 * qh : 512 * (qh + 1)],
                                    ksl,
                                    qk_t[
                                        po : po + HD,
                                        N * qft + 512 * qh : N * qft + 512 * (qh + 1),
                                    ],
                                    start=True,
                                    stop=True,
                                )
                            pt = ptp.tile([128, N], BF16, tag="pt")
                            nc.scalar.activation(pt[:], ps_s[:], EXP)
                            pts.append(pt)
                        for m in range(w0, w0 + wave):
                            v_block(m)
                        for kt in range(w0, w0 + wave):
                            vsl = vp_t[:, WV * kt : WV * kt + HD + 1]
                            for qh in range(2):
                                sl = slice(512 * qh, 512 * (qh + 1))
                                nc.tensor.matmul(
                                    ps_u[0:65, sl], vsl, pts[kt - w0][:, sl],
                                    start=(kt == 0), stop=(kt == NTT - 1),
                                )
                    normalize(po, qft, ps_u, 0, N)

                def head(h):
                    """Baseline head structure: U(kt) directly follows
                    exp(kt), with a filler chunk at most kts to cover the
                    exp latency. A force-drain guard completes any qk
                    blocks this head reads."""
                    qft, po = h // 2, 64 * (h % 2)
                    kft = 6 + h // 2
                    while filler and filler[0][0] in (qft, kft):
                        filler.popleft()[1]()
                    ps_u = psp.tile([128, N], F32, tag="u")
                    for kt in range(NTT):
                        ps_s = psp.tile([128, N], F32, tag="s")
                        ksl = qk_t[
                            po : po + HD,
                            N * kft + 128 * kt : N * kft + 128 * (kt + 1),
                        ]
                        for qh in range(2):
                            nc.tensor.matmul(
                                ps_s[:, 512 * qh : 512 * (qh + 1)],
                                ksl,
                                qk_t[
                                    po : po + HD,
                                    N * qft + 512 * qh : N * qft + 512 * (qh + 1),
                                ],
                                start=True,
                                stop=True,
                            )
                        pt = ptp.tile([128, N], BF16, tag="pt")
                        nc.scalar.activation(pt[:], ps_s[:], EXP)
                        vsl = vp_t[
                            :, WV * kt + (HD + 1) * h : WV * kt + (HD + 1) * (h + 1)
                        ]
                        for qh in range(2):
                            sl = slice(512 * qh, 512 * (qh + 1))
                            nc.tensor.matmul(
                                ps_u[0:65, sl], vsl, pt[:, sl],
                                start=(kt == 0), stop=(kt == NTT - 1),
                            )
                        if filler and (kt >= 2 or h % 2 == 1):
                            filler.popleft()[1]()
                    normalize(po, qft, ps_u, 0, N)

                def proj_m(m, tag="s", split_out=False):
                    """Output projection for token tile m + bias + DMA out."""
                    ps_o = psp.tile([128, N], F32, tag=tag, name="ps_proj")
                    for ct in range(NCT):
                        lhs = attnT[:, N * ct + 128 * m : N * ct + 128 * (m + 1)]
                        for nn, nw in ((0, 512), (512, 256)):
                            nc.tensor.matmul(
                                ps_o[:, nn : nn + nw],
                                lhs,
                                wp_t[:, C * ct + nn : C * ct + nn + nw],
                                start=(ct == 0),
                                stop=(ct == NCT - 1),
                            )
                    o_sb = wkp.tile([128, C], F32, tag="osb", bufs=4)
                    if split_out:
                        for c0, c1 in ((0, 384), (384, C)):
                            nc.vector.tensor_tensor(
                                o_sb[:, c0:c1], ps_o[:, c0:c1],
                                bias_sb[:, c0:c1], op=mybir.AluOpType.add,
                            )
                            nc.gpsimd.dma_start(
                                out_d[128 * m : 128 * (m + 1), c0:c1],
                                o_sb[:, c0:c1],
                            )
                    else:
                        nc.vector.tensor_tensor(
                            o_sb[:], ps_o[:, 0:C], bias_sb[:], op=mybir.AluOpType.add
                        )
                        nc.gpsimd.dma_start(out_d[128 * m : 128 * (m + 1), :], o_sb[:])

                def head_last(h):
                    """Last head, split into two 512-col query halves with
                    baseline-style U-after-exp inside each half. The first
                    half's proj tiles run between half B's S and U, doubling
                    as exp-latency cover; only half B's proj remains serial."""
                    qft, po = h // 2, 64 * (h % 2)
                    kft = 6 + h // 2
                    ps_u = psp.tile([128, N], F32, tag="u")

                    def sx_kt(kt, nh):
                        ps_s = psp.tile([128, N], F32, tag="s")
                        ksl = qk_t[
                            po : po + HD,
                            N * kft + 128 * kt : N * kft + 128 * (kt + 1),
                        ]
                        nc.tensor.matmul(
                            ps_s[:, 0:512],
                            ksl,
                            qk_t[po : po + HD, N * qft + 512 * nh : N * qft + 512 * (nh + 1)],
                            start=True,
                            stop=True,
                        )
                        pt = ptp.tile([128, N], BF16, tag="pt")
                        nc.scalar.activation(pt[:, 0:512], ps_s[:, 0:512], EXP)
                        return pt

                    def u_kt(kt, nh, pt):
                        sl = slice(512 * nh, 512 * (nh + 1))
                        vsl = vp_t[
                            :, WV * kt + (HD + 1) * h : WV * kt + (HD + 1) * (h + 1)
                        ]
                        nc.tensor.matmul(
                            ps_u[0:65, sl], vsl, pt[:, 0:512],
                            start=(kt == 0), stop=(kt == NTT - 1),
                        )

                    for kt in range(NTT):
                        pt = sx_kt(kt, 0)
                        if filler:
                            filler.popleft()[1]()
                        u_kt(kt, 0, pt)
                    normalize(po, qft, ps_u, 0, 512)
                    for kt in range(NTT):
                        pt = sx_kt(kt, 1)
                        if kt < 4:
                            proj_m(kt)
                        u_kt(kt, 1, pt)
                    normalize(po, qft, ps_u, 512, N)
                    for m in range(4, NTT):
                        proj_m(m, split_out=(m == NTT - 1))

                # pre-head phase: blocks 1,7 monolithic while wv streams
                qk_compute(1, w_1)
                qk_compute(7, w_7)

                # pair loads issue just-in-time (two heads ahead) so a
                # ring-slot wait never blocks the Pool queue head-of-line.
                loads = {}
                loads[1] = (wq_load(2), wq_load(8))
                for h in range(H):
                    if h in loads:
                        t = h // 2 + 2
                        wa, wb = loads.pop(h)
                        queue_qk_chunks(t, wa)
                        queue_qk_chunks(6 + t, wb)
                        if t + 1 <= 5:
                            loads[h + 2] = (wq_load(t + 1), wq_load(t + 7))
                    if h == 6:
                        nc.gpsimd.dma_start(
                            wp_t[:].rearrange("p (ct f) -> p ct f", f=C),
                            wp_d[:].rearrange("(ct p) f -> p ct f", p=128),
                        )
                    if h == 0:
                        head0_split()
                    elif h == H - 1:
                        while filler:
                            filler.popleft()[1]()
                        head_last(h)
                    else:
                        head(h)

            if reps:
                with tc.For_i(0, reps, 1):
                    emit()
            else:
                emit()

    nc.compile()
    return nc


_CACHE = {}


def _get_nc():
    if "nc" not in _CACHE:
        _CACHE["nc"] = _build()
    return _CACHE["nc"]


def _host_prep(w_qkv, w_proj, b_proj):
    import ml_dtypes

    bf = ml_dtypes.bfloat16
    ws = np.asarray(w_qkv, dtype=np.float32).copy()
    ws[0:C] *= SCALE
    wt = np.ascontiguousarray(ws.T)  # [768, 2304]
    wqb = np.ascontiguousarray(
        wt[:, : 2 * C].reshape(C, NQK, 128).transpose(1, 0, 2)
    ).astype(bf)
    wv = np.ascontiguousarray(wt[:, 2 * C :]).astype(bf)
    wp = np.ascontiguousarray(np.asarray(w_proj, dtype=np.float32).T).astype(bf)
    bp = np.ascontiguousarray(np.tile(np.asarray(b_proj, dtype=np.float32)[None, :], (128, 1)))
    return wqb, wv, wp, bp


def kernel(x, w_qkv, w_proj, b_proj):
    x = np.asarray(x, dtype=np.float32)
    assert x.shape == (B, N, C), x.shape
    wqb, wv, wp, bp = _host_prep(w_qkv, w_proj, b_proj)
    import ml_dtypes

    in_maps = [
        {
            "xT": np.ascontiguousarray(x[b].T).astype(ml_dtypes.bfloat16),
            "wqb": wqb,
            "wv": wv,
            "wp": wp,
            "bp": bp,
        }
        for b in range(B)
    ]
    nc = _get_nc()
    res = bass_utils.run_bass_kernel_spmd(nc, in_maps, core_ids=list(range(B)))
    return np.stack([np.asarray(res.results[b]["out"]) for b in range(B)]).astype(
        np.float32
    )


# revision 4
# speedup vs baseline: 1.0829x; 1.0829x over previous
"""Trainium2 Bass kernel for nn_Attention_9998683865539 (bf16 operands, baseline head order).

Multi-head attention (B=8, N=1024, C=768, H=12, HD=64, fp32), data-parallel
over the batch across 8 NeuronCores (one batch element per core, weights
replicated, no collectives).

v2 scheduling changes over the baseline:
  - DMA order: w_qk blocks 0,6 land first as single transfers, then x in
    per-ct chunks consumed immediately by interleaved qk0/qk6 matmuls, then
    blocks 1,7 (computed monolithically pre-head-0), then wv.
  - head(): U matmuls lag S by one k-tile so the PE never waits on the
    ACT exp; qk filler chunks sit between S(kt) and U(kt-1).
  - last head is split into two query halves; the output projection for
    the first half's token tiles is interleaved into the second half's
    pipeline, shrinking the serial tail.
"""
import sys

sys.path.insert(0, "/opt/trn_rl_repo")

import collections

import numpy as np

import concourse.bass as bass
import concourse.tile as tile
from concourse import bacc, mybir
from concourse import bass_utils

F32 = mybir.dt.float32
F32R = mybir.dt.float32r
BF16 = mybir.dt.bfloat16
EXP = mybir.ActivationFunctionType.Exp
MULT = mybir.AluOpType.mult

B = 8            # batch (one element per core)
C = 768          # channels
N = 1024         # tokens
H = 12           # heads
HD = 64          # head dim
SCALE = HD ** -0.5
NCT = C // 128   # 6 channel tiles
NTT = N // 128   # 8 token tiles
NQK = 12         # qk feature tiles (1536/128)
WV = H * (HD + 1)  # 780: per token-tile, 12 heads x (64 v + 1 ones)


def _build(reps=0, pt_bufs=5, wqs_bufs=6, lag=2):
    nc = bacc.Bacc("TRN2", target_bir_lowering=False, debug=False)

    xT_d = nc.dram_tensor("xT", [C, N], BF16, kind="ExternalInput").ap()
    wqb_d = nc.dram_tensor("wqb", [NQK, C, 128], BF16, kind="ExternalInput").ap()
    wv_d = nc.dram_tensor("wv", [C, C], BF16, kind="ExternalInput").ap()
    wp_d = nc.dram_tensor("wp", [C, C], BF16, kind="ExternalInput").ap()
    bp_d = nc.dram_tensor("bp", [128, C], F32, kind="ExternalInput").ap()
    out_d = nc.dram_tensor("out", [N, C], F32, kind="ExternalOutput").ap()

    with tile.TileContext(nc) as tc:
        with (
            tc.tile_pool(name="big", bufs=1) as big,
            tc.tile_pool(name="ptp", bufs=pt_bufs) as ptp,
            tc.tile_pool(name="wkp", bufs=1) as wkp,
            tc.tile_pool(name="psp", bufs=2, space=bass.MemorySpace.PSUM) as psp,
        ):
            qk_t = big.tile([128, NQK * N], F32R)     # 48KB/part (on-chip only)
            vp_t = big.tile([128, NTT * WV], F32R)    # 24.4KB/part
            attnT = big.tile([128, NCT * N], BF16)    # 12KB/part
            xr = big.tile([128, NCT * N], BF16)       # 12KB/part
            wv_t = big.tile([128, NCT * C], BF16)     # 9KB/part
            wp_t = big.tile([128, NCT * C], BF16)     # 9KB/part
            ones12 = wkp.tile([128, H], BF16)
            bias_sb = wkp.tile([128, C], F32)
            warm = wkp.tile([128, 1], F32)

            LAG = lag

            def emit():
                # constants; dummy exp pulls the ACT table load (~2.7us)
                # into the DMA ramp instead of the first real exp
                nc.vector.memset(ones12[:], 1.0)
                nc.scalar.activation(warm[:], ones12[:, 0:1], EXP)
                ps_w = psp.tile([128, N], F32, tag="s", name="ps_warm")
                for _ in range(30):
                    nc.tensor.matmul(
                        ps_w[0:H, 0:H], ones12[:], ones12[:],
                        start=True, stop=True,
                    )

                def wq_load(ft, eng=None):
                    wqs = wkp.tile(
                        [128, NCT * 128], BF16, tag="wqs", bufs=wqs_bufs
                    )
                    (eng or nc.gpsimd).dma_start(
                        wqs[:].rearrange("p (ct f) -> p ct f", f=128),
                        wqb_d[ft].rearrange("(ct p) f -> p ct f", p=128),
                    )
                    return wqs

                # DMA order: w0, w6 (whole blocks), x per-ct chunks, w1, w7,
                # wv, bias; descriptor generation is spread across the SP /
                # DVE / ACT / Pool queues so the ramp is transfer-bound.
                w_0 = wq_load(0, nc.gpsimd)
                w_6 = wq_load(6, nc.gpsimd)
                xq = [nc.sync, nc.scalar, nc.sync, nc.scalar,
                      nc.sync, nc.scalar]
                for ct in range(NCT):
                    xq[ct].dma_start(
                        xr[:, N * ct : N * (ct + 1)],
                        xT_d[128 * ct : 128 * (ct + 1), :],
                    )
                w_1 = wq_load(1, nc.gpsimd)
                w_7 = wq_load(7, nc.gpsimd)
                nc.gpsimd.dma_start(
                    wv_t[:].rearrange("p (ct f) -> p ct f", f=C),
                    wv_d[:].rearrange("(ct p) f -> p ct f", p=128),
                )
                nc.sync.dma_start(bias_sb[:], bp_d[:])

                # qk blocks 0 and 6 interleaved per-ct: consume x chunks as
                # they arrive, accumulating in two PSUM s-slots.
                ps0 = psp.tile([128, N], F32, tag="s", name="ps_qk0")
                ps6 = psp.tile([128, N], F32, tag="s", name="ps_qk6")
                for ct in range(NCT):
                    for ps, wb in ((ps0, w_0), (ps6, w_6)):
                        lhs = wb[:, 128 * ct : 128 * (ct + 1)]
                        for qh in range(2):
                            nc.tensor.matmul(
                                ps[:, 512 * qh : 512 * (qh + 1)],
                                lhs,
                                xr[:, N * ct + 512 * qh : N * ct + 512 * (qh + 1)],
                                start=(ct == 0),
                                stop=(ct == NCT - 1),
                            )
                nc.vector.tensor_copy(qk_t[:, 0:N], ps0[:])
                nc.vector.tensor_copy(qk_t[:, 6 * N : 7 * N], ps6[:])


                def qk_compute(ft, wqs):
                    """qkT block ft, monolithic (pre-head phase)."""
                    ps = psp.tile([128, N], F32, tag="s")
                    for ct in range(NCT):
                        lhs = wqs[:, 128 * ct : 128 * (ct + 1)]
                        for qh in range(2):
                            nc.tensor.matmul(
                                ps[:, 512 * qh : 512 * (qh + 1)],
                                lhs,
                                xr[:, N * ct + 512 * qh : N * ct + 512 * (qh + 1)],
                                start=(ct == 0),
                                stop=(ct == NCT - 1),
                            )
                    nc.vector.tensor_copy(qk_t[:, N * ft : N * (ft + 1)], ps[:])

                filler = collections.deque()

                def queue_qk_chunks(ft, wqs):
                    """qkT block ft as 6 filler chunks (2 matmuls each),
                    accumulating in a u-tag PSUM slot."""
                    cell = {}

                    def chunk(ct):
                        if ct == 0:
                            cell["ps"] = psp.tile(
                                [128, N], F32, tag="u", name="qk_acc"
                            )
                        ps = cell["ps"]
                        lhs = wqs[:, 128 * ct : 128 * (ct + 1)]
                        for qh in range(2):
                            nc.tensor.matmul(
                                ps[:, 512 * qh : 512 * (qh + 1)],
                                lhs,
                                xr[:, N * ct + 512 * qh : N * ct + 512 * (qh + 1)],
                                start=(ct == 0),
                                stop=(ct == NCT - 1),
                            )
                        if ct == NCT - 1:
                            nc.vector.tensor_copy(
                                qk_t[:, N * ft : N * (ft + 1)], ps[:]
                            )

                    for ct in range(NCT):
                        filler.append((ft, lambda ct=ct: chunk(ct)))

                def v_block(m, tag="s"):
                    """v token-tile m -> vp [128, 780]: 12x(64 v cols + ones)."""
                    ps = psp.tile([128, N], F32, tag=tag, name="v_acc")
                    for ct in range(NCT):
                        lhs = xr[:, N * ct + 128 * m : N * ct + 128 * (m + 1)]
                        for nn, nw in ((0, 512), (512, 256)):
                            nc.tensor.matmul(
                                ps[:, nn : nn + nw],
                                lhs,
                                wv_t[:, C * ct + nn : C * ct + nn + nw],
                                start=(ct == 0),
                                stop=(ct == NCT - 1),
                            )
                    blk = vp_t[:, WV * m : WV * (m + 1)].rearrange(
                        "p (h c) -> p h c", c=HD + 1
                    )
                    nc.vector.tensor_copy(
                        blk[:, :, 0:HD],
                        ps[:, 0:C].rearrange("p (h c) -> p h c", c=HD),
                    )
                    nc.vector.tensor_copy(
                        blk[:, :, HD : HD + 1],
                        ones12[:].rearrange("p (h o) -> p h o", o=1),
                    )

                def normalize(po, qft, ps_u, n0, n1):
                    """Evacuate U+den for cols [n0:n1), normalize off the PE:
                    recip (DVE) -> partition_broadcast (gpsimd) -> mult."""
                    w = n1 - n0
                    uT = wkp.tile([128, N], F32, tag="uT", bufs=1)
                    nc.vector.tensor_copy(uT[0:65, 0:w], ps_u[0:65, n0:n1])
                    rec_f = wkp.tile([1, N], F32, tag="recf2", bufs=1)
                    nc.vector.reciprocal(rec_f[:, 0:w], uT[64:65, 0:w])
                    bc = wkp.tile([64, N], F32, tag="bc", bufs=1)
                    nc.gpsimd.partition_broadcast(bc[:, 0:w], rec_f[:, 0:w])
                    nc.vector.tensor_tensor(
                        attnT[po : po + 64, N * qft + n0 : N * qft + n1],
                        uT[0:64, 0:w],
                        bc[:, 0:w],
                        op=MULT,
                    )

                def head0_split():
                    """Head 0 in two waves of 4 k-tiles: scores+exp emitted
                    before that wave's v blocks, so ACT drains exps while the
                    PE computes v. Wave size matches pt_bufs."""
                    qft, po, kft = 0, 0, 6
                    wave = min(pt_bufs, 4)
                    ps_u = psp.tile([128, N], F32, tag="u")
                    for w0 in range(0, NTT, wave):
                        pts = []
                        for kt in range(w0, w0 + wave):
                            ps_s = psp.tile([128, N], F32, tag="s")
                            ksl = qk_t[
                                po : po + HD,
                                N * kft + 128 * kt : N * kft + 128 * (kt + 1),
                            ]
                            for qh in range(2):
                                nc.tensor.matmul(
                                    ps_s[:, 512 * qh : 512 * (qh + 1)],
                                    ksl,
                                    qk_t[
                                        po : po + HD,
                                        N * qft + 512 * qh : N * qft + 512 * (qh + 1),
                                    ],
                                    start=True,
                                    stop=True,
                                )
                            pt = ptp.tile([128, N], F32R, tag="pt")
                            nc.scalar.activation(pt[:], ps_s[:], EXP)
                            pts.append(pt)
                        for m in range(w0, w0 + wave):
                            v_block(m)
                        for kt in range(w0, w0 + wave):
                            vsl = vp_t[:, WV * kt : WV * kt + HD + 1]
                            for qh in range(2):
                                sl = slice(512 * qh, 512 * (qh + 1))
                                nc.tensor.matmul(
                                    ps_u[0:65, sl], vsl, pts[kt - w0][:, sl],
                                    start=(kt == 0), stop=(kt == NTT - 1),
                                )
                    normalize(po, qft, ps_u, 0, N)

                def head(h):
                    """Baseline head structure: U(kt) directly follows
                    exp(kt), with a filler chunk at most kts to cover the
                    exp latency. A force-drain guard completes any qk
                    blocks this head reads."""
                    qft, po = h // 2, 64 * (h % 2)
                    kft = 6 + h // 2
                    while filler and filler[0][0] in (qft, kft):
                        filler.popleft()[1]()
                    ps_u = psp.tile([128, N], F32, tag="u")
                    for kt in range(NTT):
                        ps_s = psp.tile([128, N], F32, tag="s")
                        ksl = qk_t[
                            po : po + HD,
                            N * kft + 128 * kt : N * kft + 128 * (kt + 1),
                        ]
                        for qh in range(2):
                            nc.tensor.matmul(
                                ps_s[:, 512 * qh : 512 * (qh + 1)],
                                ksl,
                                qk_t[
                                    po : po + HD,
                                    N * qft + 512 * qh : N * qft + 512 * (qh + 1),
                                ],
                                start=True,
                                stop=True,
                            )
                        pt = ptp.tile([128, N], F32R, tag="pt")
                        nc.scalar.activation(pt[:], ps_s[:], EXP)
                        vsl = vp_t[
                            :, WV * kt + (HD + 1) * h : WV * kt + (HD + 1) * (h + 1)
                        ]
                        for qh in range(2):
                            sl = slice(512 * qh, 512 * (qh + 1))
                            nc.tensor.matmul(
                                ps_u[0:65, sl], vsl, pt[:, sl],
                                start=(kt == 0), stop=(kt == NTT - 1),
                            )
                        if filler and (kt >= 2 or h % 2 == 1):
                            filler.popleft()[1]()
                    normalize(po, qft, ps_u, 0, N)

                def proj_m(m, tag="s", split_out=False):
                    """Output projection for token tile m + bias + DMA out."""
                    ps_o = psp.tile([128, N], F32, tag=tag, name="ps_proj")
                    for ct in range(NCT):
                        lhs = attnT[:, N * ct + 128 * m : N * ct + 128 * (m + 1)]
                        for nn, nw in ((0, 512), (512, 256)):
                            nc.tensor.matmul(
                                ps_o[:, nn : nn + nw],
                                lhs,
                                wp_t[:, C * ct + nn : C * ct + nn + nw],
                                start=(ct == 0),
                                stop=(ct == NCT - 1),
                            )
                    o_sb = wkp.tile([128, C], F32, tag="osb", bufs=4)
                    if split_out:
                        for c0, c1 in ((0, 384), (384, C)):
                            nc.vector.tensor_tensor(
                                o_sb[:, c0:c1], ps_o[:, c0:c1],
                                bias_sb[:, c0:c1], op=mybir.AluOpType.add,
                            )
                            nc.gpsimd.dma_start(
                                out_d[128 * m : 128 * (m + 1), c0:c1],
                                o_sb[:, c0:c1],
                            )
                    else:
                        nc.vector.tensor_tensor(
                            o_sb[:], ps_o[:, 0:C], bias_sb[:], op=mybir.AluOpType.add
                        )
                        nc.gpsimd.dma_start(out_d[128 * m : 128 * (m + 1), :], o_sb[:])

                def head_last(h):
                    """Last head, split into two 512-col query halves with
                    baseline-style U-after-exp inside each half. The first
                    half's proj tiles run between half B's S and U, doubling
                    as exp-latency cover; only half B's proj remains serial."""
                    qft, po = h // 2, 64 * (h % 2)
                    kft = 6 + h // 2
                    ps_u = psp.tile([128, N], F32, tag="u")

                    def sx_kt(kt, nh):
                        ps_s = psp.tile([128, N], F32, tag="s")
                        ksl = qk_t[
                            po : po + HD,
                            N * kft + 128 * kt : N * kft + 128 * (kt + 1),
                        ]
                        nc.tensor.matmul(
                            ps_s[:, 0:512],
                            ksl,
                            qk_t[po : po + HD, N * qft + 512 * nh : N * qft + 512 * (nh + 1)],
                            start=True,
                            stop=True,
                        )
                        pt = ptp.tile([128, N], F32R, tag="pt")
                        nc.scalar.activation(pt[:, 0:512], ps_s[:, 0:512], EXP)
                        return pt

                    def u_kt(kt, nh, pt):
                        sl = slice(512 * nh, 512 * (nh + 1))
                        vsl = vp_t[
                            :, WV * kt + (HD + 1) * h : WV * kt + (HD + 1) * (h + 1)
                        ]
                        nc.tensor.matmul(
                            ps_u[0:65, sl], vsl, pt[:, 0:512],
                            start=(kt == 0), stop=(kt == NTT - 1),
                        )

                    for kt in range(NTT):
                        pt = sx_kt(kt, 0)
                        if filler:
                            filler.popleft()[1]()
                        u_kt(kt, 0, pt)
                    normalize(po, qft, ps_u, 0, 512)
                    for kt in range(NTT):
                        pt = sx_kt(kt, 1)
                        if kt < 4:
                            proj_m(kt)
                        u_kt(kt, 1, pt)
                    normalize(po, qft, ps_u, 512, N)
                    for m in range(4, NTT):
                        proj_m(m, split_out=(m == NTT - 1))

                # pre-head phase: blocks 1,7 monolithic while wv streams
                qk_compute(1, w_1)
                qk_compute(7, w_7)

                # pair loads issue just-in-time (two heads ahead) so a
                # ring-slot wait never blocks the Pool queue head-of-line.
                loads = {}
                loads[1] = (wq_load(2), wq_load(8))
                for h in range(H):
                    if h in loads:
                        t = h // 2 + 2
                        wa, wb = loads.pop(h)
                        queue_qk_chunks(t, wa)
                        queue_qk_chunks(6 + t, wb)
                        if t + 1 <= 5:
                            loads[h + 2] = (wq_load(t + 1), wq_load(t + 7))
                    if h == 6:
                        nc.gpsimd.dma_start(
                            wp_t[:].rearrange("p (ct f) -> p ct f", f=C),
                            wp_d[:].rearrange("(ct p) f -> p ct f", p=128),
                        )
                    if h == 0:
                        head0_split()
                    elif h == H - 1:
                        while filler:
                            filler.popleft()[1]()
                        head_last(h)
                    else:
                        head(h)

            if reps:
                with tc.For_i(0, reps, 1):
                    emit()
            else:
                emit()

    nc.compile()
    return nc


_CACHE = {}


def _get_nc():
    if "nc" not in _CACHE:
        _CACHE["nc"] = _build()
    return _CACHE["nc"]


def _host_prep(w_qkv, w_proj, b_proj):
    import ml_dtypes

    bf = ml_dtypes.bfloat16
    ws = np.asarray(w_qkv, dtype=np.float32).copy()
    ws[0:C] *= SCALE
    wt = np.ascontiguousarray(ws.T)  # [768, 2304]
    wqb = np.ascontiguousarray(
        wt[:, : 2 * C].reshape(C, NQK, 128).transpose(1, 0, 2)
    ).astype(bf)
    wv = np.ascontiguousarray(wt[:, 2 * C :]).astype(bf)
    wp = np.ascontiguousarray(np.asarray(w_proj, dtype=np.float32).T).astype(bf)
    bp = np.ascontiguousarray(np.tile(np.asarray(b_proj, dtype=np.float32)[None, :], (128, 1)))
    return wqb, wv, wp, bp


def kernel(x, w_qkv, w_proj, b_proj):
    x = np.asarray(x, dtype=np.float32)
    assert x.shape == (B, N, C), x.shape
    wqb, wv, wp, bp = _host_prep(w_qkv, w_proj, b_proj)
    import ml_dtypes

    in_maps = [
        {
            "xT": np.ascontiguousarray(x[b].T).astype(ml_dtypes.bfloat16),
            "wqb": wqb,
            "wv": wv,
            "wp": wp,
            "bp": bp,
        }
        for b in range(B)
    ]
    nc = _get_nc()
    res = bass_utils.run_bass_kernel_spmd(nc, in_maps, core_ids=list(range(B)))
    return np.stack([np.asarray(res.results[b]["out"]) for b in range(B)]).astype(
        np.float32
    )
